# revision 66
# baseline (speedup 1.0000x reference)
"""Trainium2 Bass kernel for AttentionWeightedCELoss (v14).

Full inputs in, full (scalar) output out. Data-parallel over batch: core b
processes batch b; tiny per-class partials combine on the host.

Per-core layout: class-expanded [120 = 10 classes x 12 blocks, L=22016]
(block length padded from N/12; pad pixels carry t=10 so every reduction
ignores them). Super-tiles processed in order (st4, st0..st3) so the
small 3-slot tile fills the pipeline and the tail ends on a warm full
tile. Pipeline per super-tile:

  ACT:  E = exp(S)                          (bf16)
  DVE+Pool: ES = E*S                        (split for engine balance)
  PE:   sliding-selector matmuls collapse classes -> per-pixel
        sumexp / dot stacked [120 = 12 blocks x 10 slots, 512] PSUM;
        st4's slot 0 uses a replicated selector so partitions 36:120
        hold finite (duplicate) data -> dense ops cover all 120
        partitions and no memsets are needed (one-hots are 0 there).
  DVE:  rec = 1/sumexp; ratio = dot*rec -> interleaved Q stripe
  ACT:  lse = ln(sumexp)                  -> interleaved Q stripe
  DVE:  one-hot oh[p,(g,c,j)] = (t[p,(g,j)]==c)  (4x mode)
  PE:   per-class masked sums, ONE matmul per 8-pixel group g:
        out[(c,j),(k,j')] += sum_p oh[p,(g,c,j)] * Q[p,(g,k,j')]
        (Q groups hold ratio[0:8] and lse[8:16] interleaved)

Host computes per-class counts / n_valid / target-logit sums G_c by
bincount on the raw inputs (input-only stats), and combines them with
the device's per-class ratio/lse sums into the final scalar loss
(Ent_c = Lse_c - Rat_c, CE_c = Lse_c - G_c).
"""

import numpy as np
import ml_dtypes

import concourse.bass as bass
import concourse.bacc as bacc
import concourse.tile as tile
from concourse import mybir
from concourse.bass_utils import run_bass_kernel_spmd

F32 = mybir.dt.float32
BF16 = mybir.dt.bfloat16
NP_BF16 = np.dtype(ml_dtypes.bfloat16)

B, C, H, W = 8, 10, 512, 512
N = H * W                # 262144 pixels per batch/core
BLK = 12                 # pixel blocks (partitions = C*BLK = 120)
P = C * BLK              # 120
L = 22016                # padded block length (43 * 512)
N_PAD = BLK * L          # 264192
TPB = L // 512           # 43 tiles of 512 per block
SLOTS = 10               # slot-tiles stacked per super-tile
NST = 5                  # super-tiles (slots used: 10,10,10,10,3)
SC = NST * 512           # 2560 stacked columns
GW = 8                   # pixel-columns per masked-sum group
NG = SC // GW            # 320 groups total
GPS = 512 // GW          # 64 groups per super-tile
ORDER = (4, 0, 1, 2, 3)  # super-tile processing order (st4 = 3 slots)

_CACHE = {}


def _patch_act_tables():
    # Make the combined exp+ln set the only provider of Exp and Ln so the
    # table-load inserter picks one set (avoids ~1.3us reloads).
    import concourse.bacc as _bacc
    import concourse.mybir as _mybir
    orig = _bacc.get_activation_tables
    def filtered(arch, _orig=orig):
        tabs = _orig(arch)
        key = "natural_log_exp_and_others"
        if key not in tabs:
            return tabs
        drop = {_mybir.ActivationFunctionType.Exp,
                _mybir.ActivationFunctionType.Ln}
        out = {}
        for k, v in tabs.items():
            out[k] = set(v) if k == key else (set(v) - drop)
        return out
    _bacc.get_activation_tables = filtered


_patch_act_tables()


def _consts():
    # Sliding selector: slice [120-12*t2 : 240-12*t2] has, on partition
    # (c,b) = c*12+b, a single 1 at in-slice column m = 12*t2 + b, so the
    # matmul sums the 10 classes of block b into stacked partition 12*t2+b.
    # Columns 240:360 hold the st4 slot-0 replication selector: partition
    # (c,b) also feeds every output partition m >= 36 with m % 12 == b, so
    # the 3-slot super-tile's dead partitions carry finite duplicate data.
    selc = np.zeros((P, 360), NP_BF16)
    for c in range(C):
        for b in range(BLK):
            selc[c * BLK + b, 120 + b] = 1.0
            for m in range(b, 120, 12):
                if m < 12 or m >= 36:
                    selc[c * BLK + b, 240 + m] = 1.0
    return selc


def _build():
    nc = bacc.Bacc(None, target_bir_lowering=False)
    s_d = nc.declare_dram_parameter("s", [C, N_PAD], BF16, isOutput=False)
    t_d = nc.declare_dram_parameter("t", [P, SC], BF16, isOutput=False)
    selc_d = nc.declare_dram_parameter("selc", [P, 360], BF16, isOutput=False)
    acc_d = nc.declare_dram_parameter("acc", [80, 16], F32, isOutput=True)

    sv = s_d.rearrange("c (b l) -> (c b) l", b=BLK)  # [120, 22016]

    with tile.TileContext(nc) as tc:
        with (
            tc.tile_pool(name="const", bufs=1) as constp,
            tc.tile_pool(name="sin", bufs=3) as sinp,
            tc.tile_pool(name="ein", bufs=3) as einp,
            tc.tile_pool(name="esin", bufs=3) as esinp,
            tc.tile_pool(name="big", bufs=1) as bigp,
            tc.tile_pool(name="dense", bufs=2) as densep,
            tc.tile_pool(name="accp", bufs=1) as accp,
            tc.tile_pool(name="ps", bufs=2, space=bass.MemorySpace.PSUM) as psp,
            tc.tile_pool(name="msps", bufs=1, space=bass.MemorySpace.PSUM) as msp,
        ):
            # selc rides the ACT hwdge queue so SP's s stream starts at once;
            # SP order: st4's ramp chunks, t's first 512 cols (early one-hots
            # for st4's chains), st0 in fine 1280-col chunks (the pipeline
            # fill is DMA-supply-bound: fine chunks hide the 900ns DMA-sem
            # lag), then 2560-col chunks with the t remainder slotted in.
            selc_t = constp.tile([P, 360], BF16, tag="selc")
            s4_t = sinp.tile([P, 5120], BF16, tag="sst")
            s0_t = sinp.tile([P, 5120], BF16, tag="sst")
            s1_t = sinp.tile([P, 5120], BF16, tag="sst")
            s_tiles = {4: s4_t, 0: s0_t, 1: s1_t}
            t_t = bigp.tile([P, SC], BF16, tag="tstk")
            nc.sync.dma_start(s4_t[:, 0:512], sv[:, 20480:20992])
            nc.sync.dma_start(s4_t[:, 512:1536], sv[:, 20992:22016])
            nc.sync.dma_start(selc_t[:], selc_d[:])
            nc.sync.dma_start(t_t[:, 0:512], t_d[:, 0:512])
            nc.sync.dma_start(s0_t[:, 0:1280], sv[:, 0:1280])
            nc.sync.dma_start(s0_t[:, 1280:2560], sv[:, 1280:2560])
            nc.sync.dma_start(s0_t[:, 2560:5120], sv[:, 2560:5120])
            nc.sync.dma_start(s1_t[:, 0:2560], sv[:, 5120:7680])
            nc.sync.dma_start(t_t[:, 512:1536], t_d[:, 512:1536])
            nc.sync.dma_start(s1_t[:, 2560:5120], sv[:, 7680:10240])

            q_t = bigp.tile([P, 2 * SC], BF16, tag="q")
            qv = q_t[:].rearrange("p (g two j) -> p g two j", two=2, j=GW)

            # one-hots, group-blocked: oh[p, (g, c, j)] = (t[p, g*GW+j] == c)
            # so each chain's stationary operand is one contiguous 80-col
            # slice. Real accum_out: the BIR verifier rejects the accum-less
            # form of TensorScalarPtr (the accumulator itself is unused).
            oh_t = bigp.tile([P, C * SC], BF16, tag="oh")
            oh4 = oh_t[:].rearrange("p (g c j) -> p g c j", c=C, j=GW)
            t_v = t_t[:].rearrange("p (g j) -> p g j", j=GW)
            junk_t = constp.tile([P, 32], F32, tag="junk")
            nc.vector.memset(junk_t[:], 0.0)

            OH_CUTS = (0, 64, 192, 320)  # aligned with the t DMA arrivals

            def build_oh(part):
                gs = slice(OH_CUTS[part], OH_CUTS[part + 1])
                for c in range(C):
                    nc.vector.tensor_scalar(
                        oh4[:, gs, c], t_v[:, gs], float(c), None,
                        mybir.AluOpType.is_equal, mybir.AluOpType.add,
                        accum_out=junk_t[:, part * C + c:part * C + c + 1])

            # one PSUM bank accumulates every chain matmul: out[(c,j),(k,j')]
            ms_ps = msp.tile([80, 16], F32, tag="ms")

            # PE p-state warmup: a stream of dependency-free 1-col dummy
            # matmuls spans the DMA fill window so the ramp (full clock only
            # after 3us of continuous busy) completes before real work lands
            warm_t = constp.tile([1, 64], BF16, tag="warm")
            nc.vector.memset(warm_t[:], 0.0)
            warm_ps = msp.tile([8, 64], F32, tag="warmps")
            for _ in range(48):
                nc.tensor.matmul(warm_ps[:, 0:64], warm_t[:, 0:8], warm_t[:],
                                 start=True, stop=True, skip_group_check=True)

            ps_of = {}

            rec_of = {}

            def dense_rec(k):
                # rec = 1/se, gated only on the (early) se pass
                se_ps, _ = ps_of[k]
                rec_t = densep.tile([120, 512], F32, tag="rec")
                nc.vector.reciprocal(rec_t[:], se_ps[:])
                rec_of[k] = rec_t

            def dense_ln(k):
                # lse = ln(se) -> interleaved lse stripe; se-gated, so it
                # runs early and stays off the pipeline tail
                se_ps, _ = ps_of[k]
                gs = slice(k * GPS, (k + 1) * GPS)
                nc.scalar.activation(qv[:, gs, 1], se_ps[:],
                                     mybir.ActivationFunctionType.Ln)

            def dense_ratio(k, c0, c1):
                # ratio = dot*rec -> interleaved ratio stripe (dot-gated)
                _, dot_ps = ps_of[k]
                gs = slice(k * GPS + c0 // GW, k * GPS + c1 // GW)
                nc.vector.tensor_mul(qv[:, gs, 0], dot_ps[:, c0:c1],
                                     rec_of[k][:, c0:c1])

            def chains(k, g0, g1, first=False, last=False):
                # masked per-class sums over proc-tile k's groups [g0, g1):
                # one matmul per group; rhs = interleaved (ratio, lse).
                for gl in range(g0, g1):
                    g = k * GPS + gl
                    lhsT = oh_t[:, g * C * GW:(g + 1) * C * GW]
                    nc.tensor.matmul(
                        ms_ps[:], lhsT, q_t[:, g * 16:(g + 1) * 16],
                        start=(first and gl == g0), stop=(last and gl == g1 - 1),
                        skip_group_check=True)

            es_of = {}
            nslots_of = {}

            def sel_of(st, t2):
                if st == 4 and t2 == 0:
                    return selc_t[:, 240:360]
                return selc_t[:, 120 - 12 * t2:240 - 12 * t2]

            def dot_pass(j, first_chain=False, with_chains=True,
                         burst_at=(2, 4, 6, 8)):
                # dot selectors for tile j (its ES has had a full tile
                # period to finish, so Pool latency never gates PE); the
                # previous tile's chains fill PE bubbles in spread bursts
                nslots = nslots_of[j]
                _, dot_ps = ps_of[j]
                es_t = es_of[j]
                stj = ORDER[j]
                for t2 in range(nslots):
                    nc.tensor.matmul(dot_ps[:], sel_of(stj, t2),
                                     es_t[:, t2 * 512:(t2 + 1) * 512],
                                     start=(t2 == 0), stop=(t2 == nslots - 1))
                    if with_chains and j >= 1 and t2 in burst_at:
                        q4 = burst_at.index(t2)
                        chains(j - 1, q4 * 16, (q4 + 1) * 16,
                               first=(first_chain and q4 == 0))

            for k, st in enumerate(ORDER):
                nslots = SLOTS if st != 4 else TPB - 4 * SLOTS
                nslots_of[k] = nslots
                w = nslots * 512

                if st in s_tiles:
                    s_t = s_tiles[st]
                else:
                    s_t = sinp.tile([P, w], BF16, tag="sst")
                    nc.sync.dma_start(s_t[:, 0:2560],
                                      sv[:, st * 5120:st * 5120 + 2560])
                    if k == 3:
                        nc.sync.dma_start(t_t[:, 1536:2560], t_d[:, 1536:2560])
                    nc.sync.dma_start(s_t[:, 2560:5120],
                                      sv[:, st * 5120 + 2560:st * 5120 + 5120])
                e_t = einp.tile([P, w], BF16, tag="est")
                # ramped chunks early to shorten the pipeline fill; big
                # chunks mid-stream to amortize ACT access latency; fine
                # tail chunks on the last tile so its selector drains fast
                if st == 4:
                    cuts = (0, 512, 1536)
                elif k == 1:
                    cuts = (0, 1280, 2560, 5120)
                elif k == 4:
                    cuts = (0, 2560, 3584, 4608, 5120)
                else:
                    cuts = (0, 2560, 5120)
                for h in range(len(cuts) - 1):
                    hs = slice(cuts[h], cuts[h + 1])
                    nc.scalar.activation(e_t[:, hs], s_t[:, hs],
                                         mybir.ActivationFunctionType.Exp)
                if k > 0:
                    # previous tile's ln slots in behind this tile's exp
                    # chunks (it waits on the previous se pass anyway) and
                    # lands before the chain bursts that read the stripe
                    dense_ln(k - 1)

                # se pass: gated only on exp, so recip/ln fire early and
                # the dense chain stays off the pipeline tail
                se_ps = psp.tile([120, 512], F32, tag="se")
                dot_ps = psp.tile([120, 512], F32, tag="dot")
                for t2 in range(nslots):
                    nc.tensor.matmul(se_ps[:], sel_of(st, t2),
                                     e_t[:, t2 * 512:(t2 + 1) * 512],
                                     start=(t2 == 0), stop=(t2 == nslots - 1))

                ps_of[k] = (se_ps, dot_ps)
                if k < 4:
                    dense_rec(k)

                es_t = esinp.tile([P, w], BF16, tag="esst")
                es_of[k] = es_t
                # Pool takes an early-mid slice of each tile (the lagged dot
                # pass gives it a full period of slack, and early position
                # keeps it from cascading into the drain); DVE the rest,
                # with a fine DVE-only tail on the last tile
                if st == 4:
                    es_cuts = ((0, 512, "p"), (512, 1024, "p"),
                               (1024, 1536, "v"))
                elif k == 1:
                    es_cuts = ((0, 1280, "v"), (1280, 1792, "p"),
                               (1792, 2304, "p"), (2304, 2816, "p"),
                               (2816, 5120, "v"))
                elif k == 4:
                    es_cuts = ((0, 512, "v"), (512, 1024, "p"),
                               (1024, 1536, "p"), (1536, 2560, "v"),
                               (2560, 3584, "v"), (3584, 4608, "v"),
                               (4608, 5120, "v"))
                else:
                    es_cuts = ((0, 1024, "v"), (1024, 1536, "p"),
                               (1536, 2048, "p"), (2048, 2560, "p"),
                               (2560, 3072, "p"), (3072, 5120, "v"))
                for c0, c1, eng in es_cuts:
                    eng_ns = nc.vector if eng == "v" else nc.gpsimd
                    eng_ns.tensor_mul(es_t[:, c0:c1], e_t[:, c0:c1],
                                      s_t[:, c0:c1])

                # one-hots build behind ES as their t data lands
                if 1 <= k <= 3:
                    build_oh(k - 1)

                if k == 4:
                    # emitted after the ES singles so the DVE queue drains
                    # the critical ES tail before the (less urgent) recip
                    dense_rec(k)
                    dense_ln(k)

                if k >= 1:
                    dot_pass(k - 1, first_chain=(k == 2))
                    dense_ratio(k - 1, 0, 512)

            # drain: the last tile's dot pass (chain bursts on the early,
            # fast-ready slots so the es-gated tail runs unimpeded), ratio,
            # and chains
            dot_pass(4, burst_at=(1, 2, 3, 4))
            dense_ratio(4, 0, 512)
            chains(4, 0, GPS, last=True)

            acc_t = accp.tile([80, 16], F32, tag="acc")
            nc.vector.tensor_copy(acc_t[:], ms_ps[:])
            nc.sync.dma_start(acc_d[:], acc_t[:])

    nc.compile()
    return nc


def _host_prep(logits_b, targets):
    """Per-batch device inputs. logits_b: [C,H,W] bf16 array; targets [H,W]."""
    s = np.zeros((C, N_PAD), NP_BF16)
    s[:, :N] = logits_b.reshape(C, N)

    t_pad = np.full(N_PAD, 10.0, np.float32)
    t_pad[:N] = targets.reshape(N)

    # stacked [slot*12+b, stx*512+q] for tile T = stx*10+slot < 43
    a = np.full((SLOTS, BLK, NST, 512), 10.0, np.float32)
    fb = t_pad.reshape(BLK, TPB, 512)
    for stx in range(NST):
        for slot in range(SLOTS):
            T = stx * SLOTS + slot
            if T < TPB:
                a[slot, :, stx, :] = fb[:, T, :]
    t_stk = a.reshape(P, SC)
    # rotate columns into processing order (st4 first)
    t_stk = t_stk[:, [o * 512 + i for o in ORDER for i in range(512)]]
    return s, t_stk.astype(NP_BF16)


def kernel(logits, targets):
    logits = np.asarray(logits)
    targets = np.asarray(targets)
    logits_b = logits.astype(NP_BF16)

    if "nc" not in _CACHE:
        _CACHE["nc"] = _build()
    nc = _CACHE["nc"]

    selc = _consts()
    in_maps = []
    for b in range(B):
        s, t_stk = _host_prep(logits_b[b], targets[b])
        in_maps.append({"s": s, "t": t_stk, "selc": selc})
    res = run_bass_kernel_spmd(nc, in_maps, list(range(B)))

    # input-only per-class stats on the host (counts, n_valid, target-logit
    # sums), in float64
    t_flat = targets.reshape(-1)
    valid = t_flat != 10
    tc_v = np.clip(t_flat[valid], 0, C - 1)
    counts = np.bincount(tc_v, minlength=C).astype(np.float64)
    n_valid = counts.sum()
    lg = np.moveaxis(logits.astype(np.float64), 1, -1).reshape(-1, C)
    g = np.bincount(tc_v, weights=lg[valid, tc_v], minlength=C)

    rat = np.zeros(C, np.float64)
    lse = np.zeros(C, np.float64)
    for b in range(B):
        acc = np.asarray(res.results[b]["acc"], np.float64)  # [80, 16]
        for c in range(C):
            for j in range(GW):
                rat[c] += acc[c * GW + j, j]
                lse[c] += acc[c * GW + j, GW + j]

    ent_sum = lse - rat
    ce_sum = lse - g
    has = (counts > 0) & (n_valid > 0)
    w_base = np.where(has, (n_valid - counts) / max(n_valid, 1.0), 0.0)
    ent_mean = np.where(counts > 0, ent_sum / np.maximum(counts, 1.0), 0.0)
    w = w_base * (1.0 + 0.5 * ent_mean)
    loss = (w * ce_sum).sum() / (n_valid + 1e-6)
    return np.float32(loss)
